# revision 1
# baseline (speedup 1.0000x reference)
"""CosineClassifier Trainium2 kernel.

pred[b, c] = (img[b]/||img[b]||) . (concept[b,c]/||concept[b,c]||) / TEMP

Sharding: batch (128) split across 8 cores, 16 samples/core, no comms.

Per-core plan (memory-bound: 201 MB of concept per core):
  - concept tiles stream in natural layout [class->partition, emb->free]
  - DVE tensor_tensor_reduce computes dot(concept_row, img) in ONE pass
  - ACT activation(Square, accum_out) computes ||concept_row||^2 in ONE pass
  - epilogue: rsqrt via ACT-sqrt seed + 2 Newton steps (fp32-exact),
    PE transpose to get classes contiguous for the output DMA.
"""
import sys

for _p in ('/opt/trn_rl_repo',):
    if _p not in sys.path:
        sys.path.insert(0, _p)

import numpy as np

BS, NCLS, D = 128, 4096, 768
NCORES = 8
BPC = BS // NCORES          # samples per core
P = 128
CHUNKS = NCLS // P          # 32 class-chunks of 128 per sample
TCH = 4                     # class-chunks per DMA (512 classes = 1.5 MB)
NMAC = CHUNKS // TCH
TEMP = 0.05
INV_TEMP = 1.0 / TEMP

_CACHE = {}


def _split_multiwaits(nc, mybir):
    """This toolchain's CoreV3 codegen accepts at most 1 sync-wait per
    instruction (2 for EventSemaphore); Tile sometimes attaches more.
    Move extras onto single-wait NOPs inserted just before, same engine."""
    n = 0
    for f in nc.m.functions:
        for bb in f.blocks:
            il = bb.instructions
            if not any(
                i.sync_info is not None and i.sync_info.on_wait
                and len(i.sync_info.on_wait) > 1 for i in il
            ):
                continue
            out = []
            for inst in il:
                si = inst.sync_info
                cap = 2 if isinstance(inst, mybir.InstEventSemaphore) else 1
                if si is not None and si.on_wait and len(si.on_wait) > cap:
                    waits = list(si.on_wait)
                    for k, w in enumerate(waits[cap:]):
                        out.append(mybir.InstNoOp(
                            name=f"{inst.name}-wsplit{k}",
                            engine=inst.engine,
                            sync_info=mybir.SyncInfo(on_wait=[w], on_update=[]),
                            bass_nofuse=True,
                        ))
                        n += 1
                    si.on_wait = waits[:cap]
                out.append(inst)
            bb.instructions = out
    return n


def _build():
    from concourse import bass, mybir, tile, masks

    f32 = mybir.dt.float32
    Alu = mybir.AluOpType
    Act = mybir.ActivationFunctionType

    nc = bass.Bass("TRN2", target_bir_lowering=False, debug=False, num_devices=1)
    img = nc.dram_tensor("img", [BPC, D], f32, kind="ExternalInput").ap()
    concept = nc.dram_tensor(
        "concept", [BPC, NCLS, D], f32, kind="ExternalInput").ap()
    pred = nc.dram_tensor("pred", [BPC, NCLS], f32, kind="ExternalOutput").ap()

    with tile.TileContext(nc) as tc:
        with (
            tc.tile_pool(name="big", bufs=4) as big_pool,
            tc.tile_pool(name="imgp", bufs=2) as img_pool,
            tc.tile_pool(name="scr", bufs=2) as scr_pool,
            tc.tile_pool(name="res", bufs=1) as res_pool,
            tc.tile_pool(name="epi", bufs=2) as epi_pool,
            tc.tile_pool(name="psum", bufs=2,
                         space=bass.MemorySpace.PSUM) as psum_pool,
        ):
            y_all = res_pool.tile([P, BPC * CHUNKS], f32)   # dots
            s_all = res_pool.tile([P, BPC * CHUNKS], f32)   # |c|^2 (ACT)
            # |c|^2 for the DVE-offloaded chunks (separate tile so ACT and
            # DVE never write the same tile; merged before the epilogue)
            NOFF = 3                                        # chunks/sample on DVE
            s2_all = res_pool.tile([P, BPC * NOFF], f32)
            si_all = res_pool.tile([P, BPC], f32)           # |img|^2
            identity = res_pool.tile([P, P], f32)
            masks.make_identity(nc, identity[:])

            for b in range(BPC):
                imgb = img_pool.tile([P, D], f32, tag="imgb")
                nc.sync.dma_start(imgb[:], img[b:b + 1, :].to_broadcast((P, D)))
                si_scr = scr_pool.tile([P, D], f32, tag="sqscr")
                nc.scalar.activation(
                    si_scr[:], imgb[:], Act.Square,
                    accum_out=si_all[:, b:b + 1])
                for m in range(NMAC):
                    big = big_pool.tile([P, TCH * D], f32, tag="big")
                    src = concept[b, m * TCH * P:(m + 1) * TCH * P, :] \
                        .rearrange("(t p) d -> p t d", p=P)
                    dst = big[:].rearrange("p (t d) -> p t d", t=TCH)
                    nc.sync.dma_start(dst, src)
                    for t in range(TCH):
                        col = b * CHUNKS + m * TCH + t
                        cslice = big[:, t * D:(t + 1) * D]
                        ttr_scr = scr_pool.tile([P, D], f32, tag="ttrscr")
                        nc.vector.scalar_tensor_tensor(
                            out=ttr_scr[:],
                            in0=cslice, scalar=1.0, in1=imgb[:],
                            op0=Alu.mult, op1=Alu.mult,
                            accum_out=y_all[:, col:col + 1])
                        g = m * TCH + t
                        if g >= CHUNKS - NOFF:
                            # ACT is the bottleneck engine: offload the last
                            # 3 square-sums per sample to DVE
                            sq2_scr = scr_pool.tile([P, D], f32, tag="sq2scr")
                            c2 = b * NOFF + (g - (CHUNKS - NOFF))
                            nc.vector.scalar_tensor_tensor(
                                out=sq2_scr[:], in0=cslice, scalar=1.0,
                                in1=cslice, op0=Alu.mult, op1=Alu.mult,
                                accum_out=s2_all[:, c2:c2 + 1])
                        else:
                            sq_scr = scr_pool.tile([P, D], f32, tag="sqscr")
                            nc.scalar.activation(
                                sq_scr[:], cslice, Act.Square,
                                accum_out=s_all[:, col:col + 1])

            # merge the DVE-computed square-sums into s_all's column layout
            for b in range(BPC):
                nc.vector.tensor_copy(
                    s_all[:, b * CHUNKS + CHUNKS - NOFF:(b + 1) * CHUNKS],
                    s2_all[:, b * NOFF:(b + 1) * NOFF])

            # epilogue: pred = y * rsqrt(s*si) / TEMP, classes -> contiguous
            for b in range(BPC):
                sb = s_all[:, b * CHUNKS:(b + 1) * CHUNKS]
                yb = y_all[:, b * CHUNKS:(b + 1) * CHUNKS]
                q = epi_pool.tile([P, CHUNKS], f32, tag="q")
                nc.vector.tensor_scalar_mul(q[:], sb, si_all[:, b:b + 1])
                nc.vector.tensor_scalar_max(q[:], q[:], 1e-38)
                r = epi_pool.tile([P, CHUNKS], f32, tag="r")
                nc.scalar.activation(r[:], q[:], Act.Sqrt)
                nc.vector.reciprocal(r[:], r[:])
                t1 = epi_pool.tile([P, CHUNKS], f32, tag="t1")
                for _ in range(2):  # Newton: r <- r*(1.5 - 0.5*q*r^2)
                    nc.vector.tensor_mul(t1[:], r[:], r[:])
                    nc.vector.tensor_mul(t1[:], t1[:], q[:])
                    nc.vector.tensor_scalar(
                        out=t1[:], in0=t1[:], scalar1=-0.5, scalar2=1.5,
                        op0=Alu.mult, op1=Alu.add)
                    nc.vector.tensor_mul(r[:], r[:], t1[:])
                pb = epi_pool.tile([P, CHUNKS], f32, tag="pb")
                nc.vector.tensor_mul(pb[:], yb, r[:])
                nc.vector.tensor_scalar_mul(pb[:], pb[:], INV_TEMP)
                pt = psum_pool.tile([CHUNKS, P], f32, tag="pt")
                nc.tensor.transpose(pt[:], pb[:], identity[:])
                po = epi_pool.tile([CHUNKS, P], f32, tag="po")
                nc.vector.tensor_copy(po[:], pt[:])
                nc.sync.dma_start(
                    pred[b].rearrange("(g f) -> g f", f=P), po[:])

    _split_multiwaits(nc, mybir)
    return nc


def _get_nc():
    if 'nc' not in _CACHE:
        _CACHE['nc'] = _build()
    return _CACHE['nc']


def kernel(img: np.ndarray, concept: np.ndarray, **run_kwargs) -> np.ndarray:
    from concourse import bass_utils

    img = np.ascontiguousarray(img, dtype=np.float32)
    concept = np.ascontiguousarray(concept, dtype=np.float32)
    assert img.shape == (BS, D) and concept.shape == (BS, NCLS, D)

    nc = _get_nc()
    in_maps = [
        {"img": img[i * BPC:(i + 1) * BPC],
         "concept": concept[i * BPC:(i + 1) * BPC]}
        for i in range(NCORES)
    ]
    res = bass_utils.run_bass_kernel_spmd(
        nc, in_maps, core_ids=list(range(NCORES)), **run_kwargs)
    out = np.concatenate([r["pred"] for r in res.results], axis=0)
    if run_kwargs:
        _CACHE['last_results'] = res
    return out



# revision 7
# speedup vs baseline: 1.6506x; 1.6506x over previous
"""CosineClassifier Trainium2 kernel (v2, hybrid fp16).

pred[b, c] = (img[b]/||img[b]||) . (concept[b,c]/||concept[b,c]||) / TEMP

Sharding: batch (128) split across 8 cores, 16 samples/core, no comms.

v2 strategy (memory-bound -> halve HBM bytes, then rebalance compute
across DVE/ACT/PE so every engine sits below the new DMA floor):
  - host casts inputs to fp16 (output err ~1e-3 << 2e-2 gate); concept
    HBM traffic per core drops 201MB -> 101MB (floor ~285us @ 358GB/s).
  - classes 0..1023 keep the natural layout [class->part, emb->free]:
    DVE scalar_tensor_tensor dot + ACT Square square-sum per chunk
    (both run at 1 elem/cycle/lane; ~120us each).
  - classes 1024..4095 are host-transposed per sample to [emb, class]
    so the PE can do the work: dot = matmul with a zero-padded
    stationary ([128,32], col b = img slab; zero cols contribute
    exact zeros), square-sum = matmul(ones col 16+b) over elementwise-
    squared slabs (DVE/ACT split).  Out rows 0-15 = dots, 16-31 =
    square-sums, one [32,512] psum bank per 512-class group with a
    single long accumulation group (PE out base partition must be
    32-aligned, hence the zero-padding trick).
  - rsqrt epilogue via ACT Sqrt(scale=TEMP^2) + DVE reciprocal; the
    transposed part's epilogue runs on [32,512] tiles (all samples at
    once), with one small SBUF->SBUF DMA per group to realign rows
    16-31 onto partitions 0-15.
Engine budget/core: DMA ~290us, DVE ~250us, ACT ~250us, PE ~270us.
"""
import sys

for _p in ('/opt/trn_rl_repo',):
    if _p not in sys.path:
        sys.path.insert(0, _p)

import numpy as np

BS, NCLS, D = 128, 4096, 768
NCORES = 8
BPC = BS // NCORES          # samples per core
P = 128
KN = 1024                   # natural-layout classes (per sample)
TN = KN // P                # 8 natural chunks
WT = NCLS - KN              # 3072 transposed classes
GW = 512                    # classes per psum group
NG = WT // GW               # 6 groups
NSL = D // P                # 6 emb slabs
NDBL = NSL // 2             # 3 double-slab DMAs per sample
TEMP = 0.05
T2 = TEMP * TEMP            # Sqrt(T2*q) = TEMP*sqrt(q)

# slab-square engine split: 29 of 48 double-slabs on DVE, rest on ACT
_NDVE = 29
_DVE_SQ = [((i + 1) * _NDVE) // 48 - (i * _NDVE) // 48 == 1 for i in range(48)]

_CACHE = {}


def _split_multiwaits(nc, mybir):
    """This toolchain's CoreV3 codegen accepts at most 1 sync-wait per
    instruction (2 for EventSemaphore); Tile sometimes attaches more.
    Move extras onto single-wait NOPs inserted just before, same engine."""
    n = 0
    for f in nc.m.functions:
        for bb in f.blocks:
            il = bb.instructions
            if not any(
                i.sync_info is not None and i.sync_info.on_wait
                and len(i.sync_info.on_wait) > 1 for i in il
            ):
                continue
            out = []
            for inst in il:
                si = inst.sync_info
                cap = 2 if isinstance(inst, mybir.InstEventSemaphore) else 1
                if si is not None and si.on_wait and len(si.on_wait) > cap:
                    waits = list(si.on_wait)
                    for k, w in enumerate(waits[cap:]):
                        out.append(mybir.InstNoOp(
                            name=f"{inst.name}-wsplit{k}",
                            engine=inst.engine,
                            sync_info=mybir.SyncInfo(on_wait=[w], on_update=[]),
                            bass_nofuse=True,
                        ))
                        n += 1
                    si.on_wait = waits[:cap]
                out.append(inst)
            bb.instructions = out
    return n


def _build():
    from concourse import bass, mybir, tile, masks

    f32 = mybir.dt.float32
    f16 = mybir.dt.float16
    Alu = mybir.AluOpType
    Act = mybir.ActivationFunctionType

    nc = bass.Bass("TRN2", target_bir_lowering=False, debug=False, num_devices=1)
    img16 = nc.dram_tensor("img16", [BPC, D], f16, kind="ExternalInput").ap()
    a_nat = nc.dram_tensor(
        "a_nat", [BPC, KN, D], f16, kind="ExternalInput").ap()
    b_tr = nc.dram_tensor(
        "b_tr", [BPC, D, WT], f16, kind="ExternalInput").ap()
    dstat = nc.dram_tensor(
        "dstat", [P, NSL * BPC * 32], f16, kind="ExternalInput").ap()
    qstat = nc.dram_tensor(
        "qstat", [P, BPC * 32], f16, kind="ExternalInput").ap()
    pred = nc.dram_tensor("pred", [BPC, NCLS], f32, kind="ExternalOutput").ap()

    with tile.TileContext(nc) as tc:
        with (
            tc.tile_pool(name="res", bufs=1) as res,
            tc.tile_pool(name="natp", bufs=3) as natp,
            tc.tile_pool(name="imgp", bufs=2) as imgp,
            tc.tile_pool(name="scr", bufs=2) as scr,
            tc.tile_pool(name="dblp", bufs=3) as dblp,
            tc.tile_pool(name="sqp", bufs=3) as sqp,
            tc.tile_pool(name="epi", bufs=2) as epi,
            tc.tile_pool(name="psr", bufs=1,
                         space=bass.MemorySpace.PSUM) as psr,
            tc.tile_pool(name="pst", bufs=2,
                         space=bass.MemorySpace.PSUM) as pst,
        ):
            # ---- persistent tiles ----
            y_nat = res.tile([P, BPC * TN], f32)      # natural dots
            s_nat = res.tile([P, BPC * TN], f32)      # natural |c|^2
            si_nat = res.tile([P, BPC], f32)          # |img|^2 bcast/sample
            imgq = res.tile([32, D], f16)             # img rows twice
            si32 = res.tile([32, 1], f32)             # |img|^2, partition=b
            ds_t = res.tile([P, NSL * BPC * 32], f16)  # dot stationaries
            qs_t = res.tile([P, BPC * 32], f16)       # sq stationaries
            ident = res.tile([P, P], f32)
            masks.make_identity(nc, ident[:])

            nc.sync.dma_start(ds_t[:], dstat)
            nc.sync.dma_start(qs_t[:], qstat)
            nc.sync.dma_start(imgq[0:16, :], img16)
            nc.sync.dma_start(imgq[16:32, :], img16)
            hi_scr = res.tile([32, D], f16)
            nc.scalar.activation(
                hi_scr[:], imgq[:], Act.Square, accum_out=si32[:])

            Tg = [psr.tile([32, GW], f32, name=f"Tg{g}", tag=f"Tg{g}")
                  for g in range(NG)]

            dbl_idx = 0
            for b in range(BPC):
                # ---- natural part: classes 0..KN ----
                imgb = imgp.tile([P, D], f16, tag="imgb")
                nc.sync.dma_start(
                    imgb[:], img16[b:b + 1, :].to_broadcast((P, D)))
                si_scr = scr.tile([P, D], f16, tag="siscr")
                nc.scalar.activation(
                    si_scr[:], imgb[:], Act.Square,
                    accum_out=si_nat[:, b:b + 1])
                nat = natp.tile([P, TN * D], f16, tag="nat")
                nc.sync.dma_start(
                    nat[:].rearrange("p (t d) -> p t d", t=TN),
                    a_nat[b].rearrange("(t p) d -> p t d", p=P))
                for t in range(TN):
                    col = b * TN + t
                    cs = nat[:, t * D:(t + 1) * D]
                    stt_scr = scr.tile([P, D], f16, tag="sttscr")
                    nc.vector.scalar_tensor_tensor(
                        out=stt_scr[:], in0=cs, scalar=1.0, in1=imgb[:],
                        op0=Alu.mult, op1=Alu.mult,
                        accum_out=y_nat[:, col:col + 1])
                    sq_scr = scr.tile([P, D], f16, tag="sqscr")
                    nc.scalar.activation(
                        sq_scr[:], cs, Act.Square,
                        accum_out=s_nat[:, col:col + 1])

                # ---- transposed part: classes KN..NCLS ----
                for ds in range(NDBL):
                    dbl = dblp.tile([P, 2 * WT], f16, tag="dbl")
                    nc.sync.dma_start(
                        dbl[:].rearrange("p (s w) -> p s w", s=2),
                        b_tr[b, ds * 2 * P:(ds + 1) * 2 * P, :]
                        .rearrange("(s p) w -> p s w", p=P))
                    for h in range(2):
                        s = ds * 2 + h
                        st = ds_t[:, (s * BPC + b) * 32:(s * BPC + b + 1) * 32]
                        for g in range(NG):
                            nc.tensor.matmul(
                                Tg[g][:, :], st,
                                dbl[:, h * WT + g * GW:h * WT + (g + 1) * GW],
                                start=(b == 0 and s == 0), stop=False)
                    sq = sqp.tile([P, 2 * WT], f16, tag="sq")
                    if _DVE_SQ[dbl_idx]:
                        nc.vector.tensor_mul(sq[:], dbl[:], dbl[:])
                    else:
                        nc.scalar.activation(sq[:], dbl[:], Act.Square)
                    dbl_idx += 1
                    qst = qs_t[:, b * 32:(b + 1) * 32]
                    for h in range(2):
                        s = ds * 2 + h
                        last = (b == BPC - 1 and s == NSL - 1)
                        for g in range(NG):
                            nc.tensor.matmul(
                                Tg[g][:, :], qst,
                                sq[:, h * WT + g * GW:h * WT + (g + 1) * GW],
                                start=False, stop=(last and g == NG - 1))

                # ---- natural epilogue for sample b ----
                yb = y_nat[:, b * TN:(b + 1) * TN]
                sb = s_nat[:, b * TN:(b + 1) * TN]
                qn = epi.tile([P, TN], f32, tag="qn")
                nc.vector.tensor_scalar_mul(qn[:], sb, si_nat[:, b:b + 1])
                nc.vector.tensor_scalar_max(qn[:], qn[:], 1e-30)
                rn = epi.tile([P, TN], f32, tag="rn")
                nc.scalar.activation(rn[:], qn[:], Act.Sqrt, scale=T2)
                nc.vector.reciprocal(rn[:], rn[:])
                pn = epi.tile([P, TN], f32, tag="pn")
                nc.vector.tensor_mul(pn[:], yb, rn[:])
                pt = pst.tile([TN, P], f32, tag="pt")
                nc.tensor.transpose(pt[:], pn[:], ident[:])
                po = epi.tile([TN, P], f32, tag="po")
                nc.vector.tensor_copy(po[:], pt[:])
                nc.sync.dma_start(
                    pred[b, 0:KN].rearrange("(t p) -> t p", p=P), po[:])

            # ---- transposed epilogue: all samples at once per group ----
            for g in range(NG):
                qt = epi.tile([32, GW], f32, tag="qt")
                nc.vector.tensor_scalar_mul(qt[:], Tg[g][:, :], si32[:, 0:1])
                nc.vector.tensor_scalar_max(qt[:], qt[:], 1e-30)
                rt = epi.tile([32, GW], f32, tag="rt")
                nc.scalar.activation(rt[:], qt[:], Act.Sqrt, scale=T2)
                nc.vector.reciprocal(rt[:], rt[:])
                rlo = epi.tile([16, GW], f32, tag="rlo")
                nc.sync.dma_start(rlo[:], rt[16:32, :])
                pl = epi.tile([16, GW], f32, tag="pl")
                nc.vector.tensor_mul(pl[:], Tg[g][0:16, :], rlo[:])
                nc.sync.dma_start(
                    pred[:, KN + g * GW:KN + (g + 1) * GW], pl[:])

    _split_multiwaits(nc, mybir)
    return nc


def _get_nc():
    if 'nc' not in _CACHE:
        _CACHE['nc'] = _build()
    return _CACHE['nc']


def kernel(img: np.ndarray, concept: np.ndarray, **run_kwargs) -> np.ndarray:
    from concourse import bass_utils

    img = np.ascontiguousarray(img, dtype=np.float32)
    concept = np.ascontiguousarray(concept, dtype=np.float32)
    assert img.shape == (BS, D) and concept.shape == (BS, NCLS, D)

    img16 = img.astype(np.float16)
    a_nat = np.ascontiguousarray(concept[:, :KN, :].astype(np.float16))
    b_tr = np.ascontiguousarray(
        concept[:, KN:, :].astype(np.float16).transpose(0, 2, 1))

    nc = _get_nc()
    in_maps = []
    for i in range(NCORES):
        sl = slice(i * BPC, (i + 1) * BPC)
        imgc = img16[sl]                               # [16, 768]
        # SBUF image of the zero-padded stationaries, partition-major:
        # dstat[p, (s*BPC+b)*32 + b] = img[b, s*128+p]
        dstat = np.zeros((P, NSL * BPC * 32), np.float16)
        for s in range(NSL):
            for b in range(BPC):
                dstat[:, (s * BPC + b) * 32 + b] = imgc[b, s * P:(s + 1) * P]
        qstat = np.zeros((P, BPC * 32), np.float16)
        for b in range(BPC):
            qstat[:, b * 32 + 16 + b] = 1.0
        in_maps.append({
            "img16": imgc,
            "a_nat": a_nat[sl],
            "b_tr": b_tr[sl],
            "dstat": dstat,
            "qstat": qstat,
        })
    res = bass_utils.run_bass_kernel_spmd(
        nc, in_maps, core_ids=list(range(NCORES)), **run_kwargs)
    out = np.concatenate([r["pred"] for r in res.results], axis=0)
    if run_kwargs:
        _CACHE['last_results'] = res
    return out


# revision 15
# speedup vs baseline: 1.6901x; 1.0239x over previous
"""CosineClassifier Trainium2 kernel (v2, hybrid fp16).

pred[b, c] = (img[b]/||img[b]||) . (concept[b,c]/||concept[b,c]||) / TEMP

Sharding: batch (128) split across 8 cores, 16 samples/core, no comms.

v2 strategy (memory-bound -> halve HBM bytes, then rebalance compute
across DVE/ACT/PE so every engine sits below the new DMA floor):
  - host casts inputs to fp16 (output err ~1e-3 << 2e-2 gate); concept
    HBM traffic per core drops 201MB -> 101MB (floor ~285us @ 358GB/s).
  - classes 0..1023 keep the natural layout [class->part, emb->free]:
    DVE scalar_tensor_tensor dot + ACT Square square-sum per chunk
    (both run at 1 elem/cycle/lane; ~120us each).
  - classes 1024..4095 are host-transposed per sample to [emb, class]
    so the PE can do the work: dot = matmul with a zero-padded
    stationary ([128,32], col b = img slab; zero cols contribute
    exact zeros), square-sum = matmul(ones col 16+b) over elementwise-
    squared slabs (DVE/ACT split).  Out rows 0-15 = dots, 16-31 =
    square-sums, one [32,512] psum bank per 512-class group with a
    single long accumulation group (PE out base partition must be
    32-aligned, hence the zero-padding trick).
  - rsqrt epilogue via ACT Sqrt(scale=TEMP^2) + DVE reciprocal; the
    transposed part's epilogue runs on [32,512] tiles (all samples at
    once), with one small SBUF->SBUF DMA per group to realign rows
    16-31 onto partitions 0-15.
Engine budget/core: DMA ~290us, DVE ~250us, ACT ~250us, PE ~270us.
"""
import sys

for _p in ('/opt/trn_rl_repo',):
    if _p not in sys.path:
        sys.path.insert(0, _p)

import numpy as np

BS, NCLS, D = 128, 4096, 768
NCORES = 8
BPC = BS // NCORES          # samples per core
P = 128
KN = 1024                   # natural-layout classes (per sample)
TN = KN // P                # 8 natural chunks
WT = NCLS - KN              # 3072 transposed classes
GW = 512                    # classes per psum group
NG = WT // GW               # 6 groups
NSL = D // P                # 6 emb slabs
NDBL = NSL // 2             # 3 double-slab DMAs per sample
TEMP = 0.05
T2 = TEMP * TEMP            # Sqrt(T2*q) = TEMP*sqrt(q)

# slab-square engine split: 32 of 48 double-slabs on DVE, rest on ACT
_NDVE = 32
_DVE_SQ = [((i + 1) * _NDVE) // 48 - (i * _NDVE) // 48 == 1 for i in range(48)]
_LN_INV_TEMP = float(np.log(1.0 / TEMP))

_CACHE = {}


def _split_multiwaits(nc, mybir):
    """This toolchain's CoreV3 codegen accepts at most 1 sync-wait per
    instruction (2 for EventSemaphore); Tile sometimes attaches more.
    Move extras onto single-wait NOPs inserted just before, same engine."""
    n = 0
    for f in nc.m.functions:
        for bb in f.blocks:
            il = bb.instructions
            if not any(
                i.sync_info is not None and i.sync_info.on_wait
                and len(i.sync_info.on_wait) > 1 for i in il
            ):
                continue
            out = []
            for inst in il:
                si = inst.sync_info
                cap = 2 if isinstance(inst, mybir.InstEventSemaphore) else 1
                if si is not None and si.on_wait and len(si.on_wait) > cap:
                    waits = list(si.on_wait)
                    for k, w in enumerate(waits[cap:]):
                        out.append(mybir.InstNoOp(
                            name=f"{inst.name}-wsplit{k}",
                            engine=inst.engine,
                            sync_info=mybir.SyncInfo(on_wait=[w], on_update=[]),
                            bass_nofuse=True,
                        ))
                        n += 1
                    si.on_wait = waits[:cap]
                out.append(inst)
            bb.instructions = out
    return n


def _build():
    from concourse import bass, mybir, tile, masks

    f32 = mybir.dt.float32
    f16 = mybir.dt.float16
    Alu = mybir.AluOpType
    Act = mybir.ActivationFunctionType

    nc = bass.Bass("TRN2", target_bir_lowering=False, debug=False, num_devices=1)
    img16 = nc.dram_tensor("img16", [BPC, D], f16, kind="ExternalInput").ap()
    a_nat = nc.dram_tensor(
        "a_nat", [BPC, KN, D], f16, kind="ExternalInput").ap()
    b_tr = nc.dram_tensor(
        "b_tr", [BPC, D, WT], f16, kind="ExternalInput").ap()
    dstat = nc.dram_tensor(
        "dstat", [P, NSL * BPC * 32], f16, kind="ExternalInput").ap()
    qstat = nc.dram_tensor(
        "qstat", [P, BPC * 32], f16, kind="ExternalInput").ap()
    pred = nc.dram_tensor("pred", [BPC, NCLS], f32, kind="ExternalOutput").ap()

    with tile.TileContext(nc) as tc:
        with (
            tc.tile_pool(name="res", bufs=1) as res,
            tc.tile_pool(name="natp", bufs=4) as natp,
            tc.tile_pool(name="imgp", bufs=2) as imgp,
            tc.tile_pool(name="scr", bufs=2) as scr,
            tc.tile_pool(name="dblp", bufs=4) as dblp,
            tc.tile_pool(name="sqp", bufs=4) as sqp,
            tc.tile_pool(name="epi", bufs=2) as epi,
            tc.tile_pool(name="psr", bufs=1,
                         space=bass.MemorySpace.PSUM) as psr,
            tc.tile_pool(name="pst", bufs=2,
                         space=bass.MemorySpace.PSUM) as pst,
        ):
            # ---- persistent tiles ----
            y_nat = res.tile([P, BPC * TN], f32)      # natural dots
            s_nat = res.tile([P, BPC * TN], f32)      # natural |c|^2
            si_nat = res.tile([P, BPC], f32)          # |img|^2 bcast/sample
            imgq = res.tile([32, D], f16)             # img rows twice
            si32 = res.tile([32, 1], f32)             # |img|^2, partition=b
            ds_t = res.tile([P, NSL * BPC * 32], f16)  # dot stationaries
            qs_t = res.tile([P, BPC * 32], f16)       # sq stationaries
            ident = res.tile([P, P], f32)
            masks.make_identity(nc, ident[:])
            lnb = res.tile([P, 1], f32)       # ln(1/TEMP) bias for Exp
            nc.vector.memset(lnb[:], _LN_INV_TEMP)

            nc.sync.dma_start(ds_t[:], dstat)
            nc.sync.dma_start(qs_t[:], qstat)
            nc.sync.dma_start(imgq[0:16, :], img16)
            nc.sync.dma_start(imgq[16:32, :], img16)
            hi_scr = res.tile([32, D], f16)
            nc.scalar.activation(
                hi_scr[:], imgq[:], Act.Square, accum_out=si32[:])

            Tg = [psr.tile([32, GW], f32, name=f"Tg{g}", tag=f"Tg{g}")
                  for g in range(NG)]

            dbl_idx = 0
            for b in range(BPC):
                # ---- natural part: classes 0..KN ----
                imgb = imgp.tile([P, D], f16, tag="imgb")
                nc.sync.dma_start(
                    imgb[:], img16[b:b + 1, :].to_broadcast((P, D)))
                si_scr = scr.tile([P, D], f16, tag="siscr")
                nc.scalar.activation(
                    si_scr[:], imgb[:], Act.Square,
                    accum_out=si_nat[:, b:b + 1])
                nat = natp.tile([P, TN * D], f16, tag="nat")
                nc.sync.dma_start(
                    nat[:].rearrange("p (t d) -> p t d", t=TN),
                    a_nat[b].rearrange("(t p) d -> p t d", p=P))
                for t in range(TN):
                    col = b * TN + t
                    cs = nat[:, t * D:(t + 1) * D]
                    stt_scr = scr.tile([P, D], f16, tag="sttscr")
                    nc.vector.scalar_tensor_tensor(
                        out=stt_scr[:], in0=cs, scalar=1.0, in1=imgb[:],
                        op0=Alu.mult, op1=Alu.mult,
                        accum_out=y_nat[:, col:col + 1])
                    sq_scr = scr.tile([P, D], f16, tag="sqscr")
                    nc.scalar.activation(
                        sq_scr[:], cs, Act.Square,
                        accum_out=s_nat[:, col:col + 1])

                # ---- transposed part: classes KN..NCLS ----
                for ds in range(NDBL):
                    dbl = dblp.tile([P, 2 * WT], f16, tag="dbl")
                    nc.sync.dma_start(
                        dbl[:].rearrange("p (s w) -> p s w", s=2),
                        b_tr[b, ds * 2 * P:(ds + 1) * 2 * P, :]
                        .rearrange("(s p) w -> p s w", p=P))
                    for h in range(2):
                        s = ds * 2 + h
                        st = ds_t[:, (s * BPC + b) * 32:(s * BPC + b + 1) * 32]
                        for g in range(NG):
                            nc.tensor.matmul(
                                Tg[g][:, :], st,
                                dbl[:, h * WT + g * GW:h * WT + (g + 1) * GW],
                                start=(b == 0 and s == 0), stop=False)
                    sq = sqp.tile([P, 2 * WT], f16, tag="sq")
                    if b == BPC - 1:
                        # tail: split halves across engines so the last
                        # squares finish ~2x sooner
                        nc.vector.tensor_mul(
                            sq[:, 0:WT], dbl[:, 0:WT], dbl[:, 0:WT])
                        nc.scalar.activation(
                            sq[:, WT:2 * WT], dbl[:, WT:2 * WT], Act.Square)
                    elif _DVE_SQ[dbl_idx]:
                        nc.vector.tensor_mul(sq[:], dbl[:], dbl[:])
                    else:
                        nc.scalar.activation(sq[:], dbl[:], Act.Square)
                    dbl_idx += 1
                    qst = qs_t[:, b * 32:(b + 1) * 32]
                    for h in range(2):
                        s = ds * 2 + h
                        last = (b == BPC - 1 and s == NSL - 1)
                        for g in range(NG):
                            nc.tensor.matmul(
                                Tg[g][:, :], qst,
                                sq[:, h * WT + g * GW:h * WT + (g + 1) * GW],
                                start=False, stop=(last and g == NG - 1))

                # ---- natural epilogue for sample b ----
                yb = y_nat[:, b * TN:(b + 1) * TN]
                sb = s_nat[:, b * TN:(b + 1) * TN]
                qn = epi.tile([P, TN], f32, tag="qn")
                nc.vector.tensor_scalar_mul(qn[:], sb, si_nat[:, b:b + 1])
                nc.vector.tensor_scalar_max(qn[:], qn[:], 1e-30)
                # rsqrt(q)/TEMP = exp(-0.5*ln(q) + ln(1/TEMP)); Ln/Exp/Square
                # share one ACT table set, and this avoids the slow DVE
                # iterative-divide reciprocal.
                rn = epi.tile([P, TN], f32, tag="rn")
                nc.scalar.activation(rn[:], qn[:], Act.Ln)
                nc.scalar.activation(
                    rn[:], rn[:], Act.Exp, bias=lnb[:], scale=-0.5)
                pn = epi.tile([P, TN], f32, tag="pn")
                nc.vector.tensor_mul(pn[:], yb, rn[:])
                pt = pst.tile([TN, P], f32, tag="pt")
                nc.tensor.transpose(pt[:], pn[:], ident[:])
                po = epi.tile([TN, P], f32, tag="po")
                nc.vector.tensor_copy(po[:], pt[:])
                nc.sync.dma_start(
                    pred[b, 0:KN].rearrange("(t p) -> t p", p=P), po[:])

            # ---- transposed epilogue: all samples at once per group ----
            for g in range(NG):
                qt = epi.tile([32, GW], f32, tag="qt")
                nc.vector.tensor_scalar_mul(qt[:], Tg[g][:, :], si32[:, 0:1])
                nc.vector.tensor_scalar_max(qt[:], qt[:], 1e-30)
                rt = epi.tile([32, GW], f32, tag="rt")
                nc.scalar.activation(rt[:], qt[:], Act.Ln)
                nc.scalar.activation(
                    rt[:], rt[:], Act.Exp, bias=lnb[0:32, :], scale=-0.5)
                rlo = epi.tile([16, GW], f32, tag="rlo")
                nc.sync.dma_start(rlo[:], rt[16:32, :])
                pl = epi.tile([16, GW], f32, tag="pl")
                nc.vector.tensor_mul(pl[:], Tg[g][0:16, :], rlo[:])
                nc.sync.dma_start(
                    pred[:, KN + g * GW:KN + (g + 1) * GW], pl[:])

    _split_multiwaits(nc, mybir)
    return nc


def _get_nc():
    if 'nc' not in _CACHE:
        _CACHE['nc'] = _build()
    return _CACHE['nc']


def kernel(img: np.ndarray, concept: np.ndarray, **run_kwargs) -> np.ndarray:
    from concourse import bass_utils

    img = np.ascontiguousarray(img, dtype=np.float32)
    concept = np.ascontiguousarray(concept, dtype=np.float32)
    assert img.shape == (BS, D) and concept.shape == (BS, NCLS, D)

    img16 = img.astype(np.float16)
    a_nat = np.ascontiguousarray(concept[:, :KN, :].astype(np.float16))
    b_tr = np.ascontiguousarray(
        concept[:, KN:, :].astype(np.float16).transpose(0, 2, 1))

    nc = _get_nc()
    in_maps = []
    for i in range(NCORES):
        sl = slice(i * BPC, (i + 1) * BPC)
        imgc = img16[sl]                               # [16, 768]
        # SBUF image of the zero-padded stationaries, partition-major:
        # dstat[p, (s*BPC+b)*32 + b] = img[b, s*128+p]
        dstat = np.zeros((P, NSL * BPC * 32), np.float16)
        for s in range(NSL):
            for b in range(BPC):
                dstat[:, (s * BPC + b) * 32 + b] = imgc[b, s * P:(s + 1) * P]
        qstat = np.zeros((P, BPC * 32), np.float16)
        for b in range(BPC):
            qstat[:, b * 32 + 16 + b] = 1.0
        in_maps.append({
            "img16": imgc,
            "a_nat": a_nat[sl],
            "b_tr": b_tr[sl],
            "dstat": dstat,
            "qstat": qstat,
        })
    res = bass_utils.run_bass_kernel_spmd(
        nc, in_maps, core_ids=list(range(NCORES)), **run_kwargs)
    out = np.concatenate([r["pred"] for r in res.results], axis=0)
    if run_kwargs:
        _CACHE['last_results'] = res
    return out


# revision 18
# speedup vs baseline: 1.9159x; 1.1336x over previous
"""CosineClassifier Trainium2 kernel (v2, hybrid fp16).

pred[b, c] = (img[b]/||img[b]||) . (concept[b,c]/||concept[b,c]||) / TEMP

Sharding: batch (128) split across 8 cores, 16 samples/core, no comms.

v2 strategy (memory-bound -> halve HBM bytes, then rebalance compute
across DVE/ACT/PE so every engine sits below the new DMA floor):
  - host casts inputs to fp16 (output err ~1e-3 << 2e-2 gate); concept
    HBM traffic per core drops 201MB -> 101MB (floor ~285us @ 358GB/s).
  - classes 0..1023 keep the natural layout [class->part, emb->free]:
    DVE scalar_tensor_tensor dot + ACT Square square-sum per chunk
    (both run at 1 elem/cycle/lane; ~120us each).
  - classes 1024..4095 are host-transposed per sample to [emb, class]
    so the PE can do the work: dot = matmul with a zero-padded
    stationary ([128,32], col b = img slab; zero cols contribute
    exact zeros), square-sum = matmul(ones col 16+b) over elementwise-
    squared slabs (DVE/ACT split).  Out rows 0-15 = dots, 16-31 =
    square-sums, one [32,512] psum bank per 512-class group with a
    single long accumulation group (PE out base partition must be
    32-aligned, hence the zero-padding trick).
  - rsqrt epilogue via ACT Sqrt(scale=TEMP^2) + DVE reciprocal; the
    transposed part's epilogue runs on [32,512] tiles (all samples at
    once), with one small SBUF->SBUF DMA per group to realign rows
    16-31 onto partitions 0-15.
Engine budget/core: DMA ~290us, DVE ~250us, ACT ~250us, PE ~270us.
"""
import sys

for _p in ('/opt/trn_rl_repo',):
    if _p not in sys.path:
        sys.path.insert(0, _p)

import numpy as np

BS, NCLS, D = 128, 4096, 768
NCORES = 8
BPC = BS // NCORES          # samples per core
P = 128
KN = 1024                   # natural-layout classes (per sample)
TN = KN // P                # 8 natural chunks
WT = NCLS - KN              # 3072 transposed classes
GW = 512                    # classes per psum group
NG = WT // GW               # 6 groups
NSL = D // P                # 6 emb slabs
NDBL = NSL // 2             # 3 double-slab DMAs per sample
TEMP = 0.05
T2 = TEMP * TEMP            # Sqrt(T2*q) = TEMP*sqrt(q)

# slab-square engine split: 32 of 48 double-slabs on DVE, rest on ACT
_NDVE = 32
_DVE_SQ = [((i + 1) * _NDVE) // 48 - (i * _NDVE) // 48 == 1 for i in range(48)]
_LN_INV_TEMP = float(np.log(1.0 / TEMP))

_CACHE = {}


def _split_multiwaits(nc, mybir):
    """This toolchain's CoreV3 codegen accepts at most 1 sync-wait per
    instruction (2 for EventSemaphore); Tile sometimes attaches more.
    Move extras onto single-wait NOPs inserted just before, same engine."""
    n = 0
    for f in nc.m.functions:
        for bb in f.blocks:
            il = bb.instructions
            if not any(
                i.sync_info is not None and i.sync_info.on_wait
                and len(i.sync_info.on_wait) > 1 for i in il
            ):
                continue
            out = []
            for inst in il:
                si = inst.sync_info
                cap = 2 if isinstance(inst, mybir.InstEventSemaphore) else 1
                if si is not None and si.on_wait and len(si.on_wait) > cap:
                    waits = list(si.on_wait)
                    for k, w in enumerate(waits[cap:]):
                        out.append(mybir.InstNoOp(
                            name=f"{inst.name}-wsplit{k}",
                            engine=inst.engine,
                            sync_info=mybir.SyncInfo(on_wait=[w], on_update=[]),
                            bass_nofuse=True,
                        ))
                        n += 1
                    si.on_wait = waits[:cap]
                out.append(inst)
            bb.instructions = out
    return n


def _build():
    from concourse import bass, mybir, tile, masks

    f32 = mybir.dt.float32
    f16 = mybir.dt.float16
    Alu = mybir.AluOpType
    Act = mybir.ActivationFunctionType

    nc = bass.Bass("TRN2", target_bir_lowering=False, debug=False, num_devices=1)
    img16 = nc.dram_tensor("img16", [BPC, D], f16, kind="ExternalInput").ap()
    a_nat = nc.dram_tensor(
        "a_nat", [BPC, KN, D], f16, kind="ExternalInput").ap()
    b_tr = nc.dram_tensor(
        "b_tr", [BPC, D, WT], f16, kind="ExternalInput").ap()
    dstat = nc.dram_tensor(
        "dstat", [P, NSL * BPC * 32], f16, kind="ExternalInput").ap()
    qstat = nc.dram_tensor(
        "qstat", [P, BPC * 32], f16, kind="ExternalInput").ap()
    pred = nc.dram_tensor("pred", [BPC, NCLS], f32, kind="ExternalOutput").ap()

    with tile.TileContext(nc) as tc:
        with (
            tc.tile_pool(name="res", bufs=1) as res,
            tc.tile_pool(name="natp", bufs=3) as natp,
            tc.tile_pool(name="imgp", bufs=2) as imgp,
            tc.tile_pool(name="scr", bufs=2) as scr,
            tc.tile_pool(name="dblp", bufs=4) as dblp,
            tc.tile_pool(name="sqp", bufs=6) as sqp,
            tc.tile_pool(name="epi", bufs=2) as epi,
            tc.tile_pool(name="psr", bufs=1,
                         space=bass.MemorySpace.PSUM) as psr,
            tc.tile_pool(name="pst", bufs=2,
                         space=bass.MemorySpace.PSUM) as pst,
        ):
            # ---- persistent tiles ----
            y_nat = res.tile([P, BPC * TN], f32)      # natural dots
            s_nat = res.tile([P, BPC * TN], f32)      # natural |c|^2
            si_nat = res.tile([P, BPC], f32)          # |img|^2 bcast/sample
            imgq = res.tile([32, D], f16)             # img rows twice
            si32 = res.tile([32, 1], f32)             # |img|^2, partition=b
            ds_t = res.tile([P, NSL * BPC * 32], f16)  # dot stationaries
            qs_t = res.tile([P, BPC * 32], f16)       # sq stationaries
            ident = res.tile([P, P], f32)
            masks.make_identity(nc, ident[:])
            lnb = res.tile([P, 1], f32)       # ln(1/TEMP) bias for Exp
            nc.vector.memset(lnb[:], _LN_INV_TEMP)

            nc.sync.dma_start(ds_t[:], dstat)
            nc.sync.dma_start(qs_t[:], qstat)
            nc.sync.dma_start(imgq[0:16, :], img16)
            nc.sync.dma_start(imgq[16:32, :], img16)
            hi_scr = res.tile([32, D], f16)
            nc.scalar.activation(
                hi_scr[:], imgq[:], Act.Square, accum_out=si32[:])

            Tg = [psr.tile([32, GW], f32, name=f"Tg{g}", tag=f"Tg{g}")
                  for g in range(NG)]

            def emit_sq_mms(b, sqs):
                """sq matmuls for sample b (sqs = its 3 sq tiles)."""
                qst = qs_t[:, b * 32:(b + 1) * 32]
                for ds in range(NDBL):
                    for h in range(2):
                        s = ds * 2 + h
                        last = (b == BPC - 1 and s == NSL - 1)
                        for g in range(NG):
                            nc.tensor.matmul(
                                Tg[g][:, :], qst,
                                sqs[ds][:,
                                        h * WT + g * GW:h * WT + (g + 1) * GW],
                                start=False, stop=(last and g == NG - 1))

            dbl_idx = 0
            prev_sqs = None
            for b in range(BPC):
                # ---- DMAs first: PE-feeding slabs, then natural data ----
                dbls = []
                for ds in range(NDBL):
                    dbl = dblp.tile([P, 2 * WT], f16, tag="dbl")
                    nc.sync.dma_start(
                        dbl[:].rearrange("p (s w) -> p s w", s=2),
                        b_tr[b, ds * 2 * P:(ds + 1) * 2 * P, :]
                        .rearrange("(s p) w -> p s w", p=P))
                    dbls.append(dbl)
                imgb = imgp.tile([P, D], f16, tag="imgb")
                nc.sync.dma_start(
                    imgb[:], img16[b:b + 1, :].to_broadcast((P, D)))
                nat = natp.tile([P, TN * D], f16, tag="nat")
                nc.sync.dma_start(
                    nat[:].rearrange("p (t d) -> p t d", t=TN),
                    a_nat[b].rearrange("(t p) d -> p t d", p=P))

                # ---- squares early on DVE/ACT queues ----
                sqs = []
                for ds in range(NDBL):
                    sq = sqp.tile([P, 2 * WT], f16, tag="sq")
                    if b == BPC - 1:
                        # tail: split halves across engines so the last
                        # squares finish ~2x sooner
                        nc.vector.tensor_mul(
                            sq[:, 0:WT], dbls[ds][:, 0:WT], dbls[ds][:, 0:WT])
                        nc.scalar.activation(
                            sq[:, WT:2 * WT], dbls[ds][:, WT:2 * WT],
                            Act.Square)
                    elif _DVE_SQ[dbl_idx]:
                        nc.vector.tensor_mul(sq[:], dbls[ds][:], dbls[ds][:])
                    else:
                        nc.scalar.activation(sq[:], dbls[ds][:], Act.Square)
                    dbl_idx += 1
                    sqs.append(sq)

                # ---- PE: dot matmuls for b ----
                for ds in range(NDBL):
                    for h in range(2):
                        s = ds * 2 + h
                        st = ds_t[:, (s * BPC + b) * 32:(s * BPC + b + 1) * 32]
                        for g in range(NG):
                            nc.tensor.matmul(
                                Tg[g][:, :], st,
                                dbls[ds][:,
                                         h * WT + g * GW:h * WT + (g + 1) * GW],
                                start=(b == 0 and s == 0), stop=False)

                # ---- PE: sq matmuls for the PREVIOUS sample (pipelined so
                # PE never stalls waiting on this sample's squares) ----
                if prev_sqs is not None:
                    emit_sq_mms(b - 1, prev_sqs)
                prev_sqs = sqs

                # ---- natural part: classes 0..KN ----
                si_scr = scr.tile([P, D], f16, tag="siscr")
                nc.scalar.activation(
                    si_scr[:], imgb[:], Act.Square,
                    accum_out=si_nat[:, b:b + 1])
                for t in range(TN):
                    col = b * TN + t
                    cs = nat[:, t * D:(t + 1) * D]
                    stt_scr = scr.tile([P, D], f16, tag="sttscr")
                    nc.vector.scalar_tensor_tensor(
                        out=stt_scr[:], in0=cs, scalar=1.0, in1=imgb[:],
                        op0=Alu.mult, op1=Alu.mult,
                        accum_out=y_nat[:, col:col + 1])
                    sq_scr = scr.tile([P, D], f16, tag="sqscr")
                    nc.scalar.activation(
                        sq_scr[:], cs, Act.Square,
                        accum_out=s_nat[:, col:col + 1])

                # ---- natural epilogue for sample b ----
                yb = y_nat[:, b * TN:(b + 1) * TN]
                sb = s_nat[:, b * TN:(b + 1) * TN]
                qn = epi.tile([P, TN], f32, tag="qn")
                nc.vector.tensor_scalar_mul(qn[:], sb, si_nat[:, b:b + 1])
                nc.vector.tensor_scalar_max(qn[:], qn[:], 1e-30)
                # rsqrt(q)/TEMP = exp(-0.5*ln(q) + ln(1/TEMP)); Ln/Exp/Square
                # share one ACT table set, and this avoids the slow DVE
                # iterative-divide reciprocal.
                rn = epi.tile([P, TN], f32, tag="rn")
                nc.scalar.activation(rn[:], qn[:], Act.Ln)
                nc.scalar.activation(
                    rn[:], rn[:], Act.Exp, bias=lnb[:], scale=-0.5)
                pn = epi.tile([P, TN], f32, tag="pn")
                nc.vector.tensor_mul(pn[:], yb, rn[:])
                pt = pst.tile([TN, P], f32, tag="pt")
                nc.tensor.transpose(pt[:], pn[:], ident[:])
                po = epi.tile([TN, P], f32, tag="po")
                nc.vector.tensor_copy(po[:], pt[:])
                nc.sync.dma_start(
                    pred[b, 0:KN].rearrange("(t p) -> t p", p=P), po[:])

            # ---- last sample's sq matmuls, then the epilogues ----
            emit_sq_mms(BPC - 1, prev_sqs)

            # ---- transposed epilogue: all samples at once per group ----
            for g in range(NG):
                qt = epi.tile([32, GW], f32, tag="qt")
                nc.vector.tensor_scalar_mul(qt[:], Tg[g][:, :], si32[:, 0:1])
                nc.vector.tensor_scalar_max(qt[:], qt[:], 1e-30)
                rt = epi.tile([32, GW], f32, tag="rt")
                nc.scalar.activation(rt[:], qt[:], Act.Ln)
                nc.scalar.activation(
                    rt[:], rt[:], Act.Exp, bias=lnb[0:32, :], scale=-0.5)
                rlo = epi.tile([16, GW], f32, tag="rlo")
                nc.sync.dma_start(rlo[:], rt[16:32, :])
                pl = epi.tile([16, GW], f32, tag="pl")
                nc.vector.tensor_mul(pl[:], Tg[g][0:16, :], rlo[:])
                nc.sync.dma_start(
                    pred[:, KN + g * GW:KN + (g + 1) * GW], pl[:])

    _split_multiwaits(nc, mybir)
    return nc


def _get_nc():
    if 'nc' not in _CACHE:
        _CACHE['nc'] = _build()
    return _CACHE['nc']


def kernel(img: np.ndarray, concept: np.ndarray, **run_kwargs) -> np.ndarray:
    from concourse import bass_utils

    img = np.ascontiguousarray(img, dtype=np.float32)
    concept = np.ascontiguousarray(concept, dtype=np.float32)
    assert img.shape == (BS, D) and concept.shape == (BS, NCLS, D)

    img16 = img.astype(np.float16)
    a_nat = np.ascontiguousarray(concept[:, :KN, :].astype(np.float16))
    b_tr = np.ascontiguousarray(
        concept[:, KN:, :].astype(np.float16).transpose(0, 2, 1))

    nc = _get_nc()
    in_maps = []
    for i in range(NCORES):
        sl = slice(i * BPC, (i + 1) * BPC)
        imgc = img16[sl]                               # [16, 768]
        # SBUF image of the zero-padded stationaries, partition-major:
        # dstat[p, (s*BPC+b)*32 + b] = img[b, s*128+p]
        dstat = np.zeros((P, NSL * BPC * 32), np.float16)
        for s in range(NSL):
            for b in range(BPC):
                dstat[:, (s * BPC + b) * 32 + b] = imgc[b, s * P:(s + 1) * P]
        qstat = np.zeros((P, BPC * 32), np.float16)
        for b in range(BPC):
            qstat[:, b * 32 + 16 + b] = 1.0
        in_maps.append({
            "img16": imgc,
            "a_nat": a_nat[sl],
            "b_tr": b_tr[sl],
            "dstat": dstat,
            "qstat": qstat,
        })
    res = bass_utils.run_bass_kernel_spmd(
        nc, in_maps, core_ids=list(range(NCORES)), **run_kwargs)
    out = np.concatenate([r["pred"] for r in res.results], axis=0)
    if run_kwargs:
        _CACHE['last_results'] = res
    return out
